# revision 18
# baseline (speedup 1.0000x reference)
"""Gemma2 sliding-window attention (B=1, S=4096, HID=3584, 16 Q heads / 8 KV heads,
HD=256, window 2047, tanh softcap 50) on 8 Trainium2 NeuronCores.

Sharding: tensor-parallel over heads with NO on-device collectives. Core c owns
Q heads (2c, 2c+1) and KV head c, and computes a full-shape PARTIAL of the
output projection restricted to its own 512 attention features:
    partial_c = attn[:, 512c:512c+512] @ w_o[:, 512c:512c+512].T   [S, HID] f32
The host sums the 8 partials (unshard of the sum-sharded output). This removes
the AllGather + serial o-proj tail of the previous design.

Per-core fused pipeline over 512-token tiles tt=0..7:
  A(tt): QKV projection (transposed for Q/K, straight for V) + NeoX RoPE.
  B(tt): sliding-window attention for query block tt (keys only need tiles
         <= tt, which are already computed). Tanh-softcap, no max-subtraction
         softmax, denominator via ones-row matmul, multiplicative boundary
         masks. o-proj chunks of block tt-1 are interleaved into the PV loop
         so the tensor engine never stalls on the activation engine.
All engines stay busy inside one pass; the only exposed tail is the last
o-proj block (~30 us).

PSUM (8 banks): psX bufs=3 shared by A's QKV groups, B's score tiles and C's
o-proj accumulators; psO bufs=3 for the PV accumulators; psVD bufs=2 shared by
A's V-projection groups and B's denominator row.
"""

import sys

if "/opt/trn_rl_repo" not in sys.path:
    sys.path.insert(0, "/opt/trn_rl_repo")

import numpy as np
import ml_dtypes

import concourse.bass as bass
import concourse.tile as tile
from concourse import bacc, mybir
from concourse.bass_utils import run_bass_kernel_spmd

# Problem constants (hardcoded per harness contract)
S = 4096
HID = 3584
NH, NKV, HD = 16, 8, 256
Q_SIZE = NH * HD          # 4096
SCALE = 256.0 ** -0.5     # 1/16
SOFTCAP = 50.0
WINDOW = 2048 - 1         # 2047
THETA = 10000.0

N_CORES = 8
QK_F = 2 * HD + HD        # 768 per-core transposed-qk features: [q_h0, q_h1, k]
KO = HID // 128           # 28 contraction subtiles for projections
TT = S // 512             # 8 token tiles of 512
HC = HID // 512           # 7 output-column chunks of 512
F32 = mybir.dt.float32
BF16 = mybir.dt.bfloat16

# Boundary-tile diagonal offsets (q0 - 128*kt). Interior iff 128 <= off <= 1536.
MASK_OFFS = [-384, -256, -128, 0, 1664, 1792, 1920, 2048]

_NC_CACHE = {}


def build_nc():
    nc = bacc.Bacc()

    # All inputs are host-side pre-arranged to [128 partitions, ...contiguous]
    # so every DMA is 128 large contiguous descriptors (no 256/512-byte
    # scatter packets).
    hidT_r = nc.declare_dram_parameter("hidTp", [128, 2 * TT, KO, 256], BF16,
                                       isOutput=False)
    wqkT_r = nc.declare_dram_parameter("wqkTp", [128, 3, KO, 256], BF16,
                                       isOutput=False)
    wvT_r = nc.declare_dram_parameter("wvTp", [128, KO, HD], BF16,
                                      isOutput=False)
    woT_r = nc.declare_dram_parameter("woTp", [128, 4, HID], BF16,
                                      isOutput=False)
    cosT = nc.declare_dram_parameter("cosT", [128, S], F32, isOutput=False)
    sinT = nc.declare_dram_parameter("sinT", [128, S], F32, isOutput=False)
    masks_r = nc.declare_dram_parameter("masksp", [128, 8, 512], BF16,
                                        isOutput=False)
    out = nc.declare_dram_parameter("out", [S, HID], F32, isOutput=True)

    with tile.TileContext(nc) as tc:
        with (
            tc.tile_pool(name="persist", bufs=1) as persist,
            tc.tile_pool(name="hidp", bufs=2) as hid_pool,
            tc.tile_pool(name="cs", bufs=2) as cs_pool,
            tc.tile_pool(name="qp", bufs=2) as q_pool,
            tc.tile_pool(name="rp", bufs=2) as rp_pool,
            tc.tile_pool(name="probs", bufs=6) as probs_pool,
            tc.tile_pool(name="aop", bufs=8) as ao_pool,
            tc.tile_pool(name="otp", bufs=3) as out_pool,
            tc.tile_pool(name="small", bufs=2) as small_pool,
            tc.tile_pool(name="psX", bufs=2, space="PSUM") as psX,
            tc.tile_pool(name="psO", bufs=4, space="PSUM") as psO,
            tc.tile_pool(name="psVD", bufs=2, space="PSUM") as psVD,
        ):
            # Persistent SBUF: weights, per-tile K/V, masks, ones.
            # DMA priority: wqk pair 0 + the first hidden tile are on the
            # startup-critical path; masks/wo are deferred until after A(0)
            # is emitted so they don't steal HBM bandwidth at t=0.
            wqk_sb = persist.tile([128, 3, KO, 256], BF16, tag="wqk")
            wv_sb = persist.tile([128, KO, HD], BF16, tag="wv")
            mask_sb = persist.tile([128, 8, 512], BF16, tag="mask")
            wo_sb = persist.tile([128, 4, HID], BF16, tag="wo")
            ones_sb = persist.tile([128, 1], BF16, tag="ones")
            nc.vector.memset(ones_sb, 1.0)

            k_sb = [persist.tile([128, 2, 512], BF16, tag=f"k{t}", name=f"k{t}")
                    for t in range(TT)]
            v_sb = [persist.tile([128, 4, HD], BF16, tag=f"v{t}", name=f"v{t}")
                    for t in range(TT)]

            ao_store = {}

            def prefetch(tt):
                """Issue hid/cos/sin DMAs for tile tt ahead of the out-DMA
                flood in the sync queue."""
                hids = []
                for half in range(2):
                    hid_h = hid_pool.tile([128, KO, 256], BF16, tag="hid",
                                          name="hid_h")
                    nc.sync.dma_start(hid_h, hidT_r[:, tt * 2 + half, :, :])
                    hids.append(hid_h)
                cos_t = cs_pool.tile([128, 512], F32, tag="cos", name="cos_t")
                nc.sync.dma_start(cos_t, cosT[:, bass.ts(tt, 512)])
                sin_t = cs_pool.tile([128, 512], F32, tag="sin", name="sin_t")
                nc.sync.dma_start(sin_t, sinT[:, bass.ts(tt, 512)])
                return hids, cos_t, sin_t

            def emit_A(tt, pre):
                """QKV projection + RoPE for token tile tt. Returns q tile."""
                hids, cos_t, sin_t = pre
                q_t = q_pool.tile([128, 4, 512], BF16, tag="q", name="q_t")
                for half in range(2):
                    csl = bass.ts(half, 256)
                    hid_h = hids[half]
                    for pair in range(3):
                        if tt == 0 and half == 0:
                            # just-in-time weight loads: pair p's slice only
                            # lands right before its first use, so the
                            # startup-critical pair-0 slice isn't bandwidth-
                            # starved by the later ones
                            nc.sync.dma_start(wqk_sb[:, pair, :, :],
                                              wqkT_r[:, pair, :, :])
                        ps_a = psX.tile([128, 256], F32, tag="x", name="ps_a")
                        for ko in range(KO):
                            nc.tensor.matmul(
                                ps_a,
                                wqk_sb[:, pair, ko, 0:128],
                                hid_h[:, ko, :],
                                start=(ko == 0), stop=(ko == KO - 1),
                            )
                        ps_b = psX.tile([128, 256], F32, tag="x", name="ps_b")
                        for ko in range(KO):
                            nc.tensor.matmul(
                                ps_b,
                                wqk_sb[:, pair, ko, 128:256],
                                hid_h[:, ko, :],
                                start=(ko == 0), stop=(ko == KO - 1),
                            )
                        if pair < 2:
                            d1 = q_t[:, 2 * pair, csl]
                            d2 = q_t[:, 2 * pair + 1, csl]
                        else:
                            d1 = k_sb[tt][:, 0, csl]
                            d2 = k_sb[tt][:, 1, csl]
                        t1 = rp_pool.tile([128, 256], F32, tag="rp", name="t1")
                        t2 = rp_pool.tile([128, 256], F32, tag="rp", name="t2")
                        nc.vector.tensor_mul(t1, ps_a, cos_t[:, csl])
                        nc.vector.tensor_mul(t2, ps_b, sin_t[:, csl])
                        nc.vector.tensor_sub(d1, t1, t2)
                        t3 = rp_pool.tile([128, 256], F32, tag="rp", name="t3")
                        t4 = rp_pool.tile([128, 256], F32, tag="rp", name="t4")
                        nc.vector.tensor_mul(t3, ps_b, cos_t[:, csl])
                        nc.vector.tensor_mul(t4, ps_a, sin_t[:, csl])
                        nc.vector.tensor_add(d2, t3, t4)
                    if tt == 0 and half == 0:
                        nc.sync.dma_start(wv_sb, wvT_r[:, :, :])
                    for j in range(2):
                        ps_v = psVD.tile([128, HD], F32, tag="vd", name="ps_v")
                        for ko in range(KO):
                            nc.tensor.matmul(
                                ps_v,
                                hid_h[:, ko, bass.ts(j, 128)],
                                wv_sb[:, ko, :],
                                start=(ko == 0), stop=(ko == KO - 1),
                            )
                        nc.scalar.copy(v_sb[tt][:, half * 2 + j, :], ps_v)
                return q_t

            def emit_C_chunks(qb):
                """o-proj partial for query block qb: 28 chunk generators."""
                ao_h0 = ao_store.pop((qb, 0))
                ao_h1 = ao_store.pop((qb, 1))
                aos = [ao_h0[0], ao_h0[1], ao_h1[0], ao_h1[1]]
                for tsub in range(4):
                    for hc in range(HC):
                        ps = psX.tile([128, 512], F32, tag="x", name="psC")
                        for fs in range(4):
                            nc.tensor.matmul(
                                ps,
                                aos[fs][:, bass.ts(tsub, 128)],
                                wo_sb[:, fs, bass.ts(hc, 512)],
                                start=(fs == 0), stop=(fs == 3),
                                skip_group_check=True,
                            )
                        ot = out_pool.tile([128, 512], F32, tag="ot", name="ot")
                        # alternate the PSUM->SBUF copy between ACT and DVE so
                        # neither queue saturates in the B+C window
                        if hc % 2 == 0:
                            nc.scalar.copy(ot, ps)
                        else:
                            nc.vector.tensor_scalar_add(ot, ps, 0.0)
                        r0 = qb * 512 + tsub * 128
                        nc.sync.dma_start(
                            out[r0:r0 + 128, bass.ts(hc, 512)], ot
                        )
                        yield

            def emit_B(qb, q_t, cgen):
                """Attention for query block qb, o-proj chunks interleaved."""
                q0 = qb * 512
                kts = list(range(max(0, 4 * qb - 16), 4 * qb + 4))
                n = len(kts)
                for h in range(2):
                    po0 = psO.tile([128, 512], F32, tag="po", name="po0")
                    po1 = psO.tile([128, 512], F32, tag="po", name="po1")
                    pden = psVD.tile([1, 512], F32, tag="vd", name="pden")
                    probs = {}

                    def scores(i, h=h, probs=probs):
                        kt = kts[i]
                        ttk, ksub = kt // 4, kt % 4
                        ksl = bass.ts(ksub, 128)
                        ps = psX.tile([128, 512], F32, tag="x", name="ps_s")
                        nc.tensor.matmul(
                            ps, k_sb[ttk][:, 0, ksl], q_t[:, 2 * h, :],
                            start=True, stop=False,
                        )
                        nc.tensor.matmul(
                            ps, k_sb[ttk][:, 1, ksl], q_t[:, 2 * h + 1, :],
                            start=False, stop=True,
                        )
                        pt = probs_pool.tile([128, 512], BF16, tag="pt",
                                             name="pt")
                        nc.scalar.activation(
                            ps, ps, mybir.ActivationFunctionType.Tanh,
                            scale=SCALE / SOFTCAP,
                        )
                        nc.scalar.activation(
                            pt, ps, mybir.ActivationFunctionType.Exp,
                            scale=SOFTCAP,
                        )
                        off = q0 - 128 * kt
                        if not (128 <= off <= 1536):
                            mi = MASK_OFFS.index(off)
                            nc.gpsimd.tensor_mul(pt, pt, mask_sb[:, mi, :])
                        probs[i] = pt

                    def av(i, probs=probs, po0=po0, po1=po1, pden=pden):
                        kt = kts[i]
                        ttk, ksub = kt // 4, kt % 4
                        pt = probs.pop(i)
                        st, sp = (i == 0), (i == n - 1)
                        nc.tensor.matmul(po0, v_sb[ttk][:, ksub, 0:128], pt,
                                         start=st, stop=sp,
                                         skip_group_check=True)
                        nc.tensor.matmul(po1, v_sb[ttk][:, ksub, 128:256], pt,
                                         start=st, stop=sp,
                                         skip_group_check=True)
                        nc.tensor.matmul(pden, ones_sb, pt,
                                         start=st, stop=sp,
                                         skip_group_check=True)

                    LOOK = 3
                    for i in range(min(LOOK, n)):
                        scores(i)
                    next(cgen, None)
                    budget = 0.0
                    for i in range(n):
                        if i + LOOK < n:
                            scores(i + LOOK)
                        av(i)
                        budget += 11.0 / n
                        while budget >= 1.0:
                            next(cgen, None)
                            budget -= 1.0
                    next(cgen, None)
                    next(cgen, None)

                    recip = small_pool.tile([1, 512], F32, tag="recip",
                                            name="recip")
                    nc.vector.reciprocal(recip, pden)
                    rb = small_pool.tile([128, 512], F32, tag="rb", name="rb")
                    nc.gpsimd.partition_broadcast(rb, recip)
                    ao0 = ao_pool.tile([128, 512], BF16, tag="ao", name="ao0")
                    ao1 = ao_pool.tile([128, 512], BF16, tag="ao", name="ao1")
                    nc.vector.tensor_mul(ao0, po0, rb)
                    nc.vector.tensor_mul(ao1, po1, rb)
                    ao_store[(qb, h)] = (ao0, ao1)

            pre = prefetch(0)
            for tt in range(TT):
                q_t = emit_A(tt, pre)
                if tt == 0:
                    # deferred low-priority loads (needed from B(0) / C(0) on)
                    nc.sync.dma_start(mask_sb, masks_r[:, :, :])
                    for fs in range(4):
                        nc.sync.dma_start(wo_sb[:, fs, :], woT_r[:, fs, :])
                if tt + 1 < TT:
                    pre = prefetch(tt + 1)
                cgen = emit_C_chunks(tt - 1) if tt > 0 else iter(())
                emit_B(tt, q_t, cgen)
                for _ in cgen:
                    pass
            for _ in emit_C_chunks(TT - 1):
                pass

    nc.compile()
    return nc


def get_nc():
    if "nc" not in _NC_CACHE:
        _NC_CACHE["nc"] = build_nc()
    return _NC_CACHE["nc"]


def prep_in_maps(inputs):
    bf16 = ml_dtypes.bfloat16
    hs = np.asarray(inputs["hidden_states"], dtype=np.float32)
    pos = np.asarray(inputs["position_ids"]).reshape(-1).astype(np.float64)
    w_qkv = np.asarray(inputs["w_qkv"], dtype=np.float32)
    w_o = np.asarray(inputs["w_o"], dtype=np.float32)

    # hidTp[p, th, ko, q] = hs[256*th + q, 128*ko + p]
    hidTp = np.ascontiguousarray(
        hs.reshape(2 * TT, 256, KO, 128).astype(bf16).transpose(3, 0, 2, 1)
    )

    inv_freq = 1.0 / (THETA ** (np.arange(HD // 2, dtype=np.float64) * 2.0 / HD))
    ang = inv_freq[:, None] * pos[None, :]
    cosT = np.cos(ang).astype(np.float32)
    sinT = np.sin(ang).astype(np.float32)

    kk = np.arange(128)[:, None]
    qq = np.arange(512)[None, :]
    masksp = np.stack(
        [((qq - kk + o >= 0) & (qq - kk + o <= WINDOW)) for o in MASK_OFFS],
        axis=1,
    ).astype(bf16)  # [128, 8, 512]

    in_maps = []
    for c in range(N_CORES):
        wq = w_qkv[512 * c:512 * (c + 1)]
        wk = w_qkv[Q_SIZE + HD * c:Q_SIZE + HD * (c + 1)]
        wv = w_qkv[Q_SIZE + NKV * HD + HD * c:Q_SIZE + NKV * HD + HD * (c + 1)]
        # [p, pr, ko, f2] = W[256*pr + f2, 128*ko + p]
        wqk = np.concatenate([wq, wk], 0)  # [768, HID]
        wqkTp = np.ascontiguousarray(
            wqk.reshape(3, 256, KO, 128).astype(bf16).transpose(3, 0, 2, 1))
        wvTp = np.ascontiguousarray(
            wv.reshape(HD, KO, 128).astype(bf16).transpose(2, 1, 0))
        # [p, fs, h] = w_o[h, 512*c + 128*fs + p]
        woTp = np.ascontiguousarray(
            w_o[:, 512 * c:512 * (c + 1)].T
            .reshape(4, 128, HID).astype(bf16).transpose(1, 0, 2))
        in_maps.append(
            dict(hidTp=hidTp, wqkTp=wqkTp, wvTp=wvTp, woTp=woTp,
                 cosT=cosT, sinT=sinT, masksp=masksp)
        )
    return in_maps


def run(inputs, **kwargs):
    nc = get_nc()
    in_maps = prep_in_maps(inputs)
    return run_bass_kernel_spmd(nc, in_maps, list(range(N_CORES)), **kwargs)


def gather_results(res):
    """Sum the 8 full-shape partials (unshard of sum-sharded output)."""
    acc = np.zeros((S, HID), dtype=np.float64)
    for c in range(N_CORES):
        acc += np.asarray(res.results[c]["out"], dtype=np.float64)
    return acc.astype(np.float32).reshape(1, S, HID)


def kernel(**inputs):
    res = run(inputs)
    return gather_results(res)
